# revision 6
# baseline (speedup 1.0000x reference)
"""Trainium2 Bass kernel for nn_CrossModalFusion (8-core data-parallel).

Key insight: Lq=Lk=1 so softmax(s)==1 exactly and attention collapses to
o = v = kv @ Wv^T; Q/K projections are dead code. Each cross-attention is
(Wo @ Wv) @ kv + (Wo @ bv + bo), fused into one matmul on host.

Layout: activations live transposed in SBUF as [128 part(h), 8 kt, cols] so
every matmul contracts hidden on partitions. LayerNorm stats via M=1
ones-matmuls; per-column broadcasts via K=1 matmuls. All matmul operands are
float32r (TF32-like: ~1e-4 rel err at full bf16 PE rate).
"""
import sys
sys.path.insert(0, '/opt/trn_rl_repo')
import numpy as np
import concourse.bass as bass
from concourse import bacc
import concourse.mybir as mybir
from concourse.tile import TileContext
from concourse.bass_utils import run_bass_kernel_spmd

AF = mybir.ActivationFunctionType
F32R = mybir.dt.float32r
FP32 = mybir.dt.float32
SUB = mybir.AluOpType.subtract

B, H, NL, FF = 16384, 1024, 4, 4096
NCORES = 8
NLOC = B // NCORES          # 2048 rows per core
PASSES = 4
G = NLOC // PASSES          # 512 cols per pass
KT = H // 128               # 8
CH = 8                      # FF chunks (4 f-tiles each)
EPS = 1e-5

_cache = {}


def _build_program():
    nc = bacc.Bacc('TRN2', target_bir_lowering=False, debug=False,
                   num_devices=NCORES)
    d = {}
    d['wA'] = nc.dram_tensor("wA", [8, 128, 64, 128], F32R, kind="ExternalInput").ap()
    d['w1'] = nc.dram_tensor("w1", [8, CH, 128, 32, 128], F32R, kind="ExternalInput").ap()
    d['w2'] = nc.dram_tensor("w2", [8, CH, 128, 32, 128], F32R, kind="ExternalInput").ap()
    d['vecs'] = nc.dram_tensor("vecs", [8, 128, 56], FP32, kind="ExternalInput").ap()
    d['b2c'] = nc.dram_tensor("b2c", [8, 1, 8, 128], F32R, kind="ExternalInput").ap()
    d['wG'] = nc.dram_tensor("wG", [2, 128, 64, 128], F32R, kind="ExternalInput").ap()
    d['wU1'] = nc.dram_tensor("wU1", [8, 128, 16, 128], F32R, kind="ExternalInput").ap()
    d['wU2'] = nc.dram_tensor("wU2", [128, 64, 128], F32R, kind="ExternalInput").ap()
    d['fvecs'] = nc.dram_tensor("fvecs", [128, 32], FP32, kind="ExternalInput").ap()
    d['pin'] = nc.dram_tensor("pin", [KT, 128, NLOC], F32R, kind="ExternalInput").ap()
    d['min'] = nc.dram_tensor("min", [KT, 128, NLOC], F32R, kind="ExternalInput").ap()
    d['flags'] = nc.dram_tensor("flags", [1, 3, NLOC], F32R, kind="ExternalInput").ap()
    d['ocol'] = nc.dram_tensor("ocol", [128, 1], F32R, kind="ExternalInput").ap()
    d['orow'] = nc.dram_tensor("orow", [1, 128], F32R, kind="ExternalInput").ap()
    d['pe_o'] = nc.dram_tensor("pe_o", [KT, 128, NLOC], F32R, kind="ExternalOutput").ap()
    d['me_o'] = nc.dram_tensor("me_o", [KT, 128, NLOC], F32R, kind="ExternalOutput").ap()
    d['fu_o'] = nc.dram_tensor("fu_o", [KT, 128, NLOC], F32R, kind="ExternalOutput").ap()

    with TileContext(nc) as tc:
        _emit(nc, tc, d)
    nc.compile()
    return nc


def _emit(nc, tc, d):
    from contextlib import ExitStack
    ctx = ExitStack()
    wpool = ctx.enter_context(tc.tile_pool(name="wpool", bufs=2))
    strm = ctx.enter_context(tc.tile_pool(name="strm", bufs=1))
    h1p = ctx.enter_context(tc.tile_pool(name="h1p", bufs=2))
    ln2p = ctx.enter_context(tc.tile_pool(name="ln2p", bufs=2))
    tmp = ctx.enter_context(tc.tile_pool(name="tmp", bufs=5))
    bcp = ctx.enter_context(tc.tile_pool(name="bcp", bufs=2))
    effp = ctx.enter_context(tc.tile_pool(name="effp", bufs=1))
    vecp = ctx.enter_context(tc.tile_pool(name="vecp", bufs=2))
    v1 = ctx.enter_context(tc.tile_pool(name="v1", bufs=6))
    smallp = ctx.enter_context(tc.tile_pool(name="smallp", bufs=1))
    ppm = ctx.enter_context(tc.tile_pool(name="ppm", bufs=3, space="PSUM"))
    pps = ctx.enter_context(tc.tile_pool(name="pps", bufs=1, space="PSUM"))
    ppb = ctx.enter_context(tc.tile_pool(name="ppb", bufs=2, space="PSUM"))

    ones_col = smallp.tile([128, 1], F32R, tag="ones_col")
    nc.sync.dma_start(ones_col[:], d['ocol'][:])
    ones_row = smallp.tile([1, 128], F32R, tag="ones_row")
    nc.sync.dma_start(ones_row[:], d['orow'][:])

    def bcast_into(pool, src_1xN, tag):
        ps = ppb.tile([128, G], FP32, tag="ps_bc")
        nc.tensor.matmul(ps[:], ones_row[:], src_1xN, start=True, stop=True)
        t = pool.tile([128, G], F32R, tag=tag)
        nc.vector.tensor_copy(t[:], ps[:])
        return t

    def bcast(src_1xN):
        return bcast_into(bcp, src_1xN, "bc")

    def ln_stats(x_tile):
        """x_tile [128, KT, G] f32r -> (mu, isig) [1, G] f32r."""
        ps1 = pps.tile([1, G], FP32, tag="ps_s1")
        for kt in range(KT):
            nc.tensor.matmul(ps1[:], ones_col[:], x_tile[:, kt, :],
                             start=(kt == 0), stop=(kt == KT - 1))
        mu = v1.tile([1, G], F32R, tag="v1")
        nc.vector.tensor_copy(mu[:], ps1[:])
        ps2 = pps.tile([1, G], FP32, tag="ps_s2")
        for kt in range(KT):
            sq = tmp.tile([128, G], F32R, tag="t")
            nc.scalar.activation(sq[:], x_tile[:, kt, :], AF.Square)
            nc.tensor.matmul(ps2[:], ones_col[:], sq[:],
                             start=(kt == 0), stop=(kt == KT - 1))
        e2 = v1.tile([1, G], F32R, tag="v1")
        nc.vector.tensor_copy(e2[:], ps2[:])
        msq = v1.tile([1, G], F32R, tag="v1")
        nc.vector.tensor_mul(msq[:], mu[:], mu[:])
        var = v1.tile([1, G], F32R, tag="v1")
        nc.vector.tensor_tensor(var[:], e2[:], msq[:], SUB)
        nc.vector.tensor_scalar_add(var[:], var[:], EPS)
        sd = v1.tile([1, G], F32R, tag="v1")
        nc.scalar.activation(sd[:], var[:], AF.Sqrt)
        isig = v1.tile([1, G], F32R, tag="v1")
        with nc.allow_low_precision(reason="f32r is 4-byte fp32 bits"):
            nc.vector.reciprocal(isig[:], sd[:])
        return mu, isig

    for pz in range(PASSES):
        c0 = pz * G
        fc = slice(c0, c0 + G)
        pS = strm.tile([128, KT, G], F32R, tag="pS")
        nc.sync.dma_start(pS[:], d['pin'][:, :, fc].rearrange("k p n -> p k n"))
        mS = strm.tile([128, KT, G], F32R, tag="mS")
        nc.sync.dma_start(mS[:], d['min'][:, :, fc].rearrange("k p n -> p k n"))
        flags_sb = smallp.tile([1, 3, G], F32R, tag="flags")
        nc.sync.dma_start(flags_sb[:], d['flags'][:, :, fc])
        effb = bcast_into(effp, flags_sb[0:1, 0, :], "effb")

        for s in range(8):
            dst, src = (pS, mS) if s % 2 == 0 else (mS, pS)
            vs = vecp.tile([128, 56], FP32, tag="vs")
            nc.sync.dma_start(vs[:], d['vecs'][s])
            b2s = smallp.tile([1, 8, 128], F32R, tag="b2s")
            nc.sync.dma_start(b2s[:], d['b2c'][s])

            # ---- phase A: attn (fused WoWv) + residual + LN1 ----
            wf = [wpool.tile([128, 32, 128], F32R, tag="w", name=f"wf{_i}") for _i in range(2)]
            nc.sync.dma_start(wf[0][:], d['wA'][s, :, 0:32, :])
            nc.sync.dma_start(wf[1][:], d['wA'][s, :, 32:64, :])
            for mt in range(8):
                ps = ppm.tile([128, G], FP32, tag="ps_m")
                wt = wf[mt // 4]
                for kt in range(KT):
                    nc.tensor.matmul(ps[:], wt[:, (mt % 4) * 8 + kt, :],
                                     src[:, kt, :],
                                     start=(kt == 0), stop=(kt == KT - 1))
                t1 = tmp.tile([128, G], F32R, tag="t")
                nc.scalar.activation(t1[:], ps[:], AF.Identity,
                                     bias=vs[:, mt:mt + 1])
                t2 = tmp.tile([128, G], F32R, tag="t")
                nc.vector.tensor_mul(t2[:], t1[:], effb[:])
                nc.vector.tensor_add(dst[:, mt, :], dst[:, mt, :], t2[:])
            mu, isig = ln_stats(dst)
            mu_b = bcast(mu[:])
            is_b = bcast(isig[:])
            for kt in range(KT):
                t3 = tmp.tile([128, G], F32R, tag="t")
                nc.vector.tensor_tensor(t3[:], dst[:, kt, :], mu_b[:], SUB)
                t4 = tmp.tile([128, G], F32R, tag="t")
                nc.vector.tensor_mul(t4[:], t3[:], is_b[:])
                nc.scalar.activation(dst[:, kt, :], t4[:], AF.Identity,
                                     scale=vs[:, 8 + kt:8 + kt + 1],
                                     bias=vs[:, 16 + kt:16 + kt + 1])

            # ---- phase B: FFN (pre-LN materialized, chunked over FF) ----
            mu2, isig2 = ln_stats(dst)
            mu2_b = bcast(mu2[:])
            is2_b = bcast(isig2[:])
            lt = ln2p.tile([128, KT, G], F32R, tag="ln2")
            for kt in range(KT):
                t5 = tmp.tile([128, G], F32R, tag="t")
                nc.vector.tensor_tensor(t5[:], dst[:, kt, :], mu2_b[:], SUB)
                nc.vector.tensor_mul(lt[:, kt, :], t5[:], is2_b[:])

            for c in range(CH):
                w1c = wpool.tile([128, 32, 128], F32R, tag="w")
                nc.sync.dma_start(w1c[:], d['w1'][s, c])
                w2c = wpool.tile([128, 32, 128], F32R, tag="w")
                nc.sync.dma_start(w2c[:], d['w2'][s, c])
                h1t = h1p.tile([128, 4, G], F32R, tag="h1")
                for ftl in range(4):
                    psf = ppm.tile([128, G], FP32, tag="ps_m")
                    for kt in range(KT):
                        nc.tensor.matmul(psf[:], w1c[:, ftl * 8 + kt, :],
                                         lt[:, kt, :],
                                         start=(kt == 0), stop=(kt == KT - 1))
                    hg = tmp.tile([128, G], F32R, tag="t")
                    cb = 24 + c * 4 + ftl
                    nc.scalar.activation(hg[:], psf[:], AF.Gelu,
                                         bias=vs[:, cb:cb + 1])
                    nc.vector.tensor_mul(h1t[:, ftl, :], hg[:], effb[:])
                for mt in range(8):
                    psm = ppm.tile([128, G], FP32, tag="ps_m")
                    for fkl in range(4):
                        last = (fkl == 3) and (c != CH - 1)
                        nc.tensor.matmul(psm[:], w2c[:, mt * 4 + fkl, :],
                                         h1t[:, fkl, :],
                                         start=(fkl == 0), stop=last)
                    if c == CH - 1:
                        nc.tensor.matmul(psm[:], b2s[0:1, mt, :],
                                         flags_sb[0:1, 0, :],
                                         start=False, stop=True)
                    nc.vector.tensor_add(dst[:, mt, :], dst[:, mt, :], psm[:])

        # ---- final: gating (stream -> cp in place; pe/me -> ln2p) + fusion ----
        fvs = vecp.tile([128, 32], FP32, tag="fvs")
        nc.sync.dma_start(fvs[:], d['fvecs'][:])
        opb = bcast(flags_sb[0:1, 1, :])
        omb = bcast(flags_sb[0:1, 2, :])
        pe_me = []
        for mod in range(2):
            st = pS if mod == 0 else mS
            orig_d = d['pin'] if mod == 0 else d['min']
            # stream := cp = stream - orig (in place, per k-tile)
            for kt in range(KT):
                po = tmp.tile([128, G], F32R, tag="t")
                nc.sync.dma_start(po[:], orig_d[kt, :, fc])
                nc.vector.tensor_tensor(st[:, kt, :], st[:, kt, :], po[:], SUB)
            wg = [wpool.tile([128, 32, 128], F32R, tag="w", name=f"wg{_i}") for _i in range(2)]
            nc.sync.dma_start(wg[0][:], d['wG'][mod, :, 0:32, :])
            nc.sync.dma_start(wg[1][:], d['wG'][mod, :, 32:64, :])
            out = ln2p.tile([128, KT, G], F32R, tag="ln2")
            for mt in range(8):
                ps = ppm.tile([128, G], FP32, tag="ps_m")
                wt = wg[mt // 4]
                for kt in range(KT):
                    nc.tensor.matmul(ps[:], wt[:, (mt % 4) * 8 + kt, :],
                                     st[:, kt, :],
                                     start=(kt == 0), stop=(kt == KT - 1))
                gate = tmp.tile([128, G], F32R, tag="t")
                nc.scalar.activation(gate[:], ps[:], AF.Sigmoid,
                                     bias=fvs[:, mod * 8 + mt:mod * 8 + mt + 1])
                ge = tmp.tile([128, G], F32R, tag="t")
                nc.vector.tensor_mul(ge[:], gate[:], effb[:])
                gc = tmp.tile([128, G], F32R, tag="t")
                nc.vector.tensor_mul(gc[:], ge[:], st[:, mt, :])
                po2 = tmp.tile([128, G], F32R, tag="t")
                nc.sync.dma_start(po2[:], orig_d[mt, :, fc])
                nc.vector.tensor_add(out[:, mt, :], po2[:], gc[:])
            out_d = d['pe_o'] if mod == 0 else d['me_o']
            nc.sync.dma_start(out_d[:, :, fc].rearrange("k p n -> p k n"),
                              out[:, :, :])
            pe_me.append(out)

        # fusion: h2 = gelu([pe;me] @ fu1T + b1) stored into pS columns
        h2 = pS  # streams are dead now; reuse as h2 home
        for mt in range(8):
            u1 = wpool.tile([128, 32, 128], F32R, tag="w")
            nc.sync.dma_start(u1[:, 0:16, :], d['wU1'][mt])
            ps = ppm.tile([128, G], FP32, tag="ps_m")
            for kt in range(16):
                stt = pe_me[0] if kt < 8 else pe_me[1]
                nc.tensor.matmul(ps[:], u1[:, kt, :], stt[:, kt % 8, :],
                                 start=(kt == 0), stop=(kt == 15))
            nc.scalar.activation(h2[:, mt, :], ps[:], AF.Gelu,
                                 bias=fvs[:, 16 + mt:16 + mt + 1])
        wu2 = [wpool.tile([128, 32, 128], F32R, tag="w", name=f"wu2{_i}") for _i in range(2)]
        nc.sync.dma_start(wu2[0][:], d['wU2'][:, 0:32, :])
        nc.sync.dma_start(wu2[1][:], d['wU2'][:, 32:64, :])
        for mt in range(8):
            ps = ppm.tile([128, G], FP32, tag="ps_m")
            wt = wu2[mt // 4]
            for kt in range(KT):
                nc.tensor.matmul(ps[:], wt[:, (mt % 4) * 8 + kt, :],
                                 h2[:, kt, :],
                                 start=(kt == 0), stop=(kt == KT - 1))
            fu = tmp.tile([128, G], F32R, tag="t")
            nc.scalar.activation(fu[:], ps[:], AF.Identity,
                                 bias=fvs[:, 24 + mt:24 + mt + 1])
            f1 = tmp.tile([128, G], F32R, tag="t")
            nc.vector.tensor_mul(f1[:], fu[:], effb[:])
            f2 = tmp.tile([128, G], F32R, tag="t")
            nc.vector.tensor_mul(f2[:], pe_me[0][:, mt, :], opb[:])
            nc.vector.tensor_add(f1[:], f1[:], f2[:])
            f3 = tmp.tile([128, G], F32R, tag="t")
            nc.vector.tensor_mul(f3[:], pe_me[1][:, mt, :], omb[:])
            nc.vector.tensor_add(f1[:], f1[:], f3[:])
            nc.sync.dma_start(d['fu_o'][mt, :, fc], f1[:])
    ctx.close()


def _prep_host(protein_repr, metabolite_repr, has_protein, has_metabolite, params):
    P = {k: np.asarray(v, dtype=np.float32) for k, v in params.items()}
    f64 = np.float64
    wA = np.zeros((8, 128, 64, 128), np.float32)
    w1 = np.zeros((8, CH, 128, 32, 128), np.float32)
    w2 = np.zeros((8, CH, 128, 32, 128), np.float32)
    vecs = np.zeros((8, 128, 56), np.float32)
    b2c = np.zeros((8, 1, 8, 128), np.float32)
    for s in range(8):
        i, pre = s // 2, ('p2m' if s % 2 == 0 else 'm2p')
        Wqkv = P[pre + '_Wqkv'][i]
        Wv, bv = Wqkv[2 * H:], P[pre + '_bqkv'][i][2 * H:]
        Wo, bo = P[pre + '_Wo'][i], P[pre + '_bo'][i]
        Wf = (Wo.astype(f64) @ Wv.astype(f64)).astype(np.float32)
        bf = (Wo.astype(f64) @ bv.astype(f64) + bo).astype(np.float32)
        g1, b1_ = P[pre + '_ng'][i], P[pre + '_nb'][i]
        fg, fb = P[pre + '_fng'][i], P[pre + '_fnb'][i]
        W1 = P[pre + '_W1'][i]
        W1p = (W1 * fg[None, :]).astype(np.float32)
        cb1 = (P[pre + '_b1'][i] + W1.astype(f64) @ fb.astype(f64)).astype(np.float32)
        W2m, b2 = P[pre + '_W2'][i], P[pre + '_b2'][i]
        wA[s] = Wf.T.reshape(8, 128, 8, 128).transpose(1, 2, 0, 3).reshape(128, 64, 128)
        w1[s] = W1p.T.reshape(8, 128, 8, 4, 128).transpose(2, 1, 3, 0, 4).reshape(8, 128, 32, 128)
        w2[s] = W2m.T.reshape(8, 4, 128, 8, 128).transpose(0, 2, 3, 1, 4).reshape(8, 128, 32, 128)
        vecs[s, :, 0:8] = bf.reshape(8, 128).T
        vecs[s, :, 8:16] = g1.reshape(8, 128).T
        vecs[s, :, 16:24] = b1_.reshape(8, 128).T
        vecs[s, :, 24:56] = cb1.reshape(32, 128).T
        b2c[s, 0] = b2.reshape(8, 128)
    wG = np.zeros((2, 128, 64, 128), np.float32)
    wG[0] = P['gp_W'].T.reshape(8, 128, 8, 128).transpose(1, 2, 0, 3).reshape(128, 64, 128)
    wG[1] = P['gm_W'].T.reshape(8, 128, 8, 128).transpose(1, 2, 0, 3).reshape(128, 64, 128)
    wU1 = P['fu_W1'].T.reshape(16, 128, 8, 128).transpose(2, 1, 0, 3).astype(np.float32).copy()
    wU2 = P['fu_W2'].T.reshape(8, 128, 8, 128).transpose(1, 2, 0, 3).reshape(128, 64, 128).copy()
    fvecs = np.zeros((128, 32), np.float32)
    fvecs[:, 0:8] = P['gp_b'].reshape(8, 128).T
    fvecs[:, 8:16] = P['gm_b'].reshape(8, 128).T
    fvecs[:, 16:24] = P['fu_b1'].reshape(8, 128).T
    fvecs[:, 24:32] = P['fu_b2'].reshape(8, 128).T

    shared = dict(wA=wA, w1=w1, w2=w2, vecs=vecs, b2c=b2c, wG=wG, wU1=wU1,
                  wU2=wU2, fvecs=fvecs,
                  ocol=np.full((128, 1), 1.0 / H, np.float32),
                  orow=np.ones((1, 128), np.float32))
    pr = np.asarray(protein_repr, np.float32)
    mr = np.asarray(metabolite_repr, np.float32)
    hp = np.asarray(has_protein, np.float32)
    hm = np.asarray(has_metabolite, np.float32)
    in_maps = []
    for c in range(NCORES):
        r = slice(c * NLOC, (c + 1) * NLOC)
        m = dict(shared)
        m['pin'] = pr[r].T.reshape(KT, 128, NLOC).copy()
        m['min'] = mr[r].T.reshape(KT, 128, NLOC).copy()
        fl = np.zeros((1, 3, NLOC), np.float32)
        fl[0, 0] = hp[r] * hm[r]
        fl[0, 1] = hp[r] * (1.0 - hm[r])
        fl[0, 2] = (1.0 - hp[r]) * hm[r]
        m['flags'] = fl
        in_maps.append(m)
    return in_maps


def kernel(protein_repr, metabolite_repr, has_protein, has_metabolite, params,
           _trace=False):
    if 'nc' not in _cache:
        _cache['nc'] = _build_program()
    nc = _cache['nc']
    in_maps = _prep_host(protein_repr, metabolite_repr, has_protein,
                         has_metabolite, params)
    res = run_bass_kernel_spmd(nc, in_maps, list(range(NCORES)), trace=_trace)
    if _trace:
        print(f"HW exec time: {res.exec_time_ns} ns")
    pe = np.empty((B, H), np.float32)
    me = np.empty((B, H), np.float32)
    fu = np.empty((B, H), np.float32)
    for c in range(NCORES):
        r = slice(c * NLOC, (c + 1) * NLOC)
        rr = res.results[c]
        pe[r] = rr['pe_o'].reshape(H, NLOC).T
        me[r] = rr['me_o'].reshape(H, NLOC).T
        fu[r] = rr['fu_o'].reshape(H, NLOC).T
    return pe, me, fu
